# revision 26
# baseline (speedup 1.0000x reference)
"""Trainium2 Bass kernel for nn_AttentionBlock (sliding-window attention block).

Computes, for full inputs x[B=2, S=2048, Hd=1024]:
    Q = rmsnorm(x @ Wq.T + bq) * qn_w   (per head, head_dim D=64)
    K = rmsnorm(x @ Wk.T + bk) * kn_w
    V = x @ Wv.T + bv
    attn = softmax(mask(Q K^T / sqrt(D)))   (causal sliding window 256)
    out = (attn @ V) @ Wo.T + bo
Returns (out [B,S,Hd] f32, attn [B,H,S,S] f32) matching the reference.

Sharding: 8 cores = 2 batches x 4 head-groups (4 heads each).  Per core:
 - Q/K projections computed on-device in transposed layout [dims, seq]
   (full-rate float32r matmuls), RMS norm in that layout with PE
   ones-matmul partition sums; V in natural layout (bf16).
 - Sliding-window logits computed transposed S_T[keys, queries], one
   128-key chunk at a time (single N=384 matmul each).  Softmax column
   sums accumulate per query-block across the 3 contributing key chunks
   via PE ones-matmul PSUM accumulation; normalization via DVE
   reciprocal + per-slice multiplies.
 - attn band stored to HBM transposed in bf16:
   band[4 heads, 16 chunks, 128 keys, 384 queries]  (6.3MB vs 64MB dense)
 - AV accumulates ctx_T[dims, seq] in PSUM (bf16 matmuls), then the
   output projection runs from ctx_T; per-core partial out written f32.
Host: shards/transposes inputs, assembles dense attn from the bands,
sums the per-core output partials, adds bo.
"""

import sys

if "/opt/trn_rl_repo" not in sys.path:
    sys.path.insert(0, "/opt/trn_rl_repo")

import numpy as np
import ml_dtypes

import concourse.bass as bass
import concourse.mybir as mybir
import concourse.tile as tile
from concourse import bacc

# problem constants (hardcoded per harness contract)
B, S, Hd = 2, 2048, 1024
H, D, WIN = 16, 64, 256
NCORES = 8
HPC = H // 4          # heads per core = 4
DPC = HPC * D         # head dims per core = 256
NBLK = S // 128       # 16 key chunks / query blocks
EPS = 1e-6

F32 = mybir.dt.float32
F32R = mybir.dt.float32r
BF16 = mybir.dt.bfloat16

_compiled = None  # cache: (key, nc)


def _r(ap, dt=F32R):
    """bitcast an AP for full-rate f32r matmuls"""
    return ap.bitcast(dt)


def build_program(has_bias_q, has_bias_k, has_bias_v, uniform_mask=True, debug_taps=False):
    AL = mybir.AluOpType
    nc = bacc.Bacc("TRN2", target_bir_lowering=False, debug=False)

    # ---------------- DRAM I/O ----------------
    xT_d = nc.dram_tensor("xT", [8, 128, S], BF16, kind="ExternalInput")      # x.T k-chunked
    wqT_d = nc.dram_tensor("wqT", [8, 128, DPC], BF16, kind="ExternalInput")  # Wq[sl].T k-chunked
    wkT_d = nc.dram_tensor("wkT", [8, 128, DPC], BF16, kind="ExternalInput")
    wvT_d = nc.dram_tensor("wvT", [8, 128, DPC], BF16, kind="ExternalInput")
    woT_d = nc.dram_tensor("woT", [2, 128, Hd], BF16, kind="ExternalInput")   # Wo[:, sl].T
    qn_d = nc.dram_tensor("qn_col", [2, 128], F32, kind="ExternalInput")     # tile(qn_w,4)/sqrt(D)
    kn_d = nc.dram_tensor("kn_col", [2, 128], F32, kind="ExternalInput")     # tile(kn_w,4)
    bq_d = nc.dram_tensor("bq_col", [2, 128], F32, kind="ExternalInput")
    bk_d = nc.dram_tensor("bk_col", [2, 128], F32, kind="ExternalInput")
    bv_d = nc.dram_tensor("bv_row", [128, DPC], F32, kind="ExternalInput")   # replicated rows
    NMSK = 1 if uniform_mask else 16
    mask_d = nc.dram_tensor("maskT", [NMSK, 128, 384], BF16, kind="ExternalInput")

    band_d = nc.dram_tensor("band", [HPC, NBLK, 128, 384], BF16, kind="ExternalOutput")
    rsum_d = nc.dram_tensor("rsum", [HPC, NBLK, 128], F32, kind="ExternalOutput")
    outp_d = nc.dram_tensor("outp", [NBLK, 128, Hd], BF16, kind="ExternalOutput")
    if debug_taps:
        qnT_d = nc.dram_tensor("dbg_qnT", [2, 128, S], F32, kind="ExternalOutput")
        knT_d = nc.dram_tensor("dbg_knT", [2, 128, S], F32, kind="ExternalOutput")
        v_d = nc.dram_tensor("dbg_v", [128, NBLK, DPC], BF16, kind="ExternalOutput")
        ctxT_dbg = nc.dram_tensor("dbg_ctxT", [2, 128, S], F32, kind="ExternalOutput")

    with tile.TileContext(nc) as tc:
        with (
            tc.tile_pool(name="const", bufs=1) as const,
            tc.tile_pool(name="wpool", bufs=1) as wpool,
            tc.tile_pool(name="xpool", bufs=1) as xpool,
            tc.tile_pool(name="qkv", bufs=1) as qkv,
            tc.tile_pool(name="normtmp", bufs=3) as normtmp,
            tc.tile_pool(name="attn_sb", bufs=4) as attn_sb,
            tc.tile_pool(name="out_sb", bufs=3) as out_sb,
            tc.tile_pool(name="ps", bufs=1, space="PSUM") as ps,
        ):
            # ---------------- constants ----------------
            ones_bd = const.tile([128, 128], F32)    # block-diag lhsT: per-head sumsq, 1/D
            nc.vector.memset(ones_bd[:], 0.0)
            nc.vector.memset(ones_bd[0:64, 0:64], 1.0 / D)
            nc.vector.memset(ones_bd[64:128, 64:128], 1.0 / D)
            ones_bf = const.tile([128, 128], BF16)   # lhsT for softmax colsums
            nc.vector.memset(ones_bf[:], 1.0)
            eps_sb = const.tile([128, 1], F32)       # rmsnorm epsilon (Sqrt bias)
            nc.vector.memset(eps_sb[:], EPS)

            # per-dim scales/biases: [128 partitions, 2 halves]
            qn_sb = const.tile([128, 2], F32)
            nc.sync.dma_start(qn_sb[:], qn_d.ap().rearrange("a b -> b a"))
            kn_sb = const.tile([128, 2], F32)
            nc.sync.dma_start(kn_sb[:], kn_d.ap().rearrange("a b -> b a"))
            if has_bias_q:
                bq_sb = const.tile([128, 2], F32)
                nc.sync.dma_start(bq_sb[:], bq_d.ap().rearrange("a b -> b a"))
            if has_bias_k:
                bk_sb = const.tile([128, 2], F32)
                nc.sync.dma_start(bk_sb[:], bk_d.ap().rearrange("a b -> b a"))
            if has_bias_v:
                bv_row = const.tile([128, DPC], F32)
                nc.sync.dma_start(bv_row[:], bv_d[:])

            # ------- weights + x, chunk-interleaved so the first matmuls start early ----
            wq_t = wpool.tile([128, 8, DPC], BF16)
            x_t = xpool.tile([128, 8, S], BF16)
            for k in range(8):
                nc.sync.dma_start(wq_t[:, k, :], wqT_d[k])
                nc.sync.dma_start(x_t[:, k, :], xT_d[k])
            wk_t = wpool.tile([128, 8, DPC], BF16)
            nc.sync.dma_start(wk_t[:], wkT_d.ap().rearrange("k p m -> p k m"))
            wv_t = wpool.tile([128, 8, DPC], BF16)
            nc.sync.dma_start(wv_t[:], wvT_d.ap().rearrange("k p m -> p k m"))
            wo_t = wpool.tile([128, 2, Hd], BF16)
            nc.sync.dma_start(wo_t[:], woT_d.ap().rearrange("k p m -> p k m"))
            mask_t = const.tile([128, NMSK, 384], BF16)
            nc.sync.dma_start(mask_t[:], mask_d.ap().rearrange("c p m -> p c m"))

            # ---------------- persistent QKV / ctx (transposed, 2 halves) ----------------
            qnT = [qkv.tile([128, S], F32, name=f"qnT{i}", tag=f"qnT{i}") for i in range(2)]
            knT = [qkv.tile([128, S], F32, name=f"knT{i}", tag=f"knT{i}") for i in range(2)]
            v_bf = qkv.tile([128, NBLK, DPC], BF16, tag="v_bf")   # V natural, s-blocked
            ctxT = [qkv.tile([128, S], BF16, name=f"ctxT{i}", tag=f"ctxT{i}") for i in range(2)]
            rsum_sb = qkv.tile([64, NBLK, 128], F32, tag="rsum_sb")  # 1/colsums per (qb)
            pm_ring = qkv.tile([128, 4, 384], BF16, tag="pm_ring")   # rolling Pm tiles

            # ========= projections as emission units (interleaved with attention) =========
            def v_proj(blk):
                pt = ps.tile([128, 512], F32, tag="projsums", bufs=4,
                             name=f"vp_{blk}")
                for k in range(8):
                    nc.tensor.matmul(
                        pt[:, :DPC],
                        x_t[:, k, blk * 128:(blk + 1) * 128],
                        wv_t[:, k, :],
                        start=(k == 0),
                        stop=(k == 7),
                    )
                if has_bias_v:
                    nc.vector.tensor_add(v_bf[:, blk, :], pt[:, :DPC], bv_row[:])
                else:
                    nc.scalar.copy(v_bf[:, blk, :], pt[:, :DPC])

            def qk_proj(which, half, sc):
                w_t, nrm_sb, b_sb, hasb, dstT = (
                    (wq_t, qn_sb, bq_sb if has_bias_q else None, has_bias_q, qnT)
                    if which == "q" else
                    (wk_t, kn_sb, bk_sb if has_bias_k else None, has_bias_k, knT))
                pt = ps.tile([128, 512], F32, tag="projsums", bufs=4,
                             name=f"pp_{which}_{half}_{sc}")
                for k in range(8):
                    nc.tensor.matmul(
                        pt[:],
                        w_t[:, k, half * 128:(half + 1) * 128],
                        x_t[:, k, sc * 512:(sc + 1) * 512],
                        start=(k == 0),
                        stop=(k == 7),
                    )
                if hasb:
                    raw = normtmp.tile([128, 512], F32, tag="raw")
                    nc.vector.tensor_scalar_add(raw[:], pt[:], b_sb[:, half:half + 1])
                    srct = raw
                else:
                    srct = pt
                sq = normtmp.tile([128, 512], F32, tag="sq")
                nc.scalar.square(sq[:], srct[:])
                sums = ps.tile([128, 512], F32, tag="mixA", bufs=2, name=f"ns_{which}_{half}_{sc}")
                nc.tensor.matmul(sums[:], _r(ones_bd[:]), _r(sq[:]), start=True, stop=True)
                rsd = normtmp.tile([128, 512], F32, tag="rsd")
                nc.scalar.activation(rsd[:], sums[:], mybir.ActivationFunctionType.Sqrt,
                                     bias=eps_sb[:, :])
                rinv = normtmp.tile([128, 512], F32, tag="rinv")
                nc.vector.reciprocal_approx_fast(rinv[:], rsd[:])
                nc.vector.scalar_tensor_tensor(
                    dstT[half][:, sc * 512:(sc + 1) * 512],
                    srct[:], nrm_sb[:, half:half + 1], rinv[:],
                    op0=AL.mult, op1=AL.mult)

            # phase 0: everything head 0 needs
            for sc in range(4):
                qk_proj("q", 0, sc)
            for sc in range(4):
                qk_proj("k", 0, sc)
            for blk in range(8):
                v_proj(blk)
            for sc in range(4):
                qk_proj("q", 1, sc)
            for sc in range(4):
                qk_proj("k", 1, sc)
            for blk in range(8, 16):
                v_proj(blk)

            # ================ attention ================
            # Per head: logits chunk ck computed transposed, exp'd, masked; its
            # unnormalized band tile goes straight to HBM.  Softmax colsums for
            # query block qb accumulate in PSUM across its 3 source chunks.
            # Normalization / AV / out-proj for qb run ONE ITERATION LATER so the
            # chain S_T->exp->mask->colsum->recip->AV pipelines across iterations.
            for a in range(HPC):  # head within core
                half, r0 = a // 2, (a % 2) * 64
                pm_tiles, sums_tiles = {}, {}

                def do_qb(qb):
                    # reciprocal of the completed colsums, straight into rsum_sb
                    nc.vector.reciprocal_approx_fast(
                        rsum_sb[:, qb, :], sums_tiles.pop(qb)[:64, :128])
                    # AV with unnormalized Pm; normalization fused into eviction
                    lo = max(0, qb - 2)
                    ct = ps.tile([64, 128], F32, tag="ctx", bufs=2,
                                 name=f"ct_{a}_{qb}")
                    for c2 in range(lo, qb + 1):
                        nc.tensor.matmul(
                            ct[:],
                            _r(v_bf[:, c2, a * 64:(a + 1) * 64], BF16),
                            _r(pm_tiles[c2][:, (qb - c2) * 128:(qb - c2 + 1) * 128], BF16),
                            start=(c2 == lo),
                            stop=(c2 == qb),
                        )
                        if c2 == qb - 2:
                            pm_tiles.pop(c2)
                    nc.vector.tensor_mul(
                        ctxT[half][r0:r0 + 64, qb * 128:(qb + 1) * 128],
                        ct[:], rsum_sb[:, qb, :])
                    # out-proj for block qb once the LAST head's ctx landed
                    if a == HPC - 1:
                        ot = out_sb.tile([128, Hd], BF16, tag="ot")
                        for nh in range(2):
                            po = ps.tile([128, 512], F32, tag="projsums", bufs=4,
                                         name=f"po_{qb}_{nh}")
                            for kh in range(2):
                                nc.tensor.matmul(
                                    po[:],
                                    ctxT[kh][:, qb * 128:(qb + 1) * 128],
                                    wo_t[:, kh, nh * 512:(nh + 1) * 512],
                                    start=(kh == 0),
                                    stop=(kh == 1),
                                )
                            (nc.scalar.copy if nh == 0 else nc.vector.tensor_copy)(
                                ot[:, nh * 512:(nh + 1) * 512], po[:])
                        nc.sync.dma_start(outp_d[qb], ot[:])

                for ck in range(NBLK):
                    n_j = min(3, NBLK - ck)
                    N = 128 * n_j
                    # S_T[keys of chunk ck, queries of blocks ck..ck+n_j-1]
                    st = ps.tile([128, 384], F32, tag="mixA", bufs=2)
                    nc.tensor.matmul(
                        st[:, :N],
                        _r(knT[half][r0:r0 + 64, ck * 128:(ck + 1) * 128]),
                        _r(qnT[half][r0:r0 + 64, ck * 128: ck * 128 + N]),
                        start=True, stop=True,
                    )
                    pexp = attn_sb.tile([128, 384], BF16, tag="pexp")
                    nc.scalar.activation(
                        pexp[:, :N], st[:, :N], mybir.ActivationFunctionType.Exp)
                    pm = pm_ring[:, ck % 4, :]
                    pm_tiles[ck] = pm
                    (nc.gpsimd if ck % 2 == 0 else nc.vector).tensor_mul(pm[:, :N], pexp[:, :N], mask_t[:, 0 if NMSK == 1 else ck, :N])
                    if N < 384:  # edge tiles: zero-fill unused region for the pair DMA
                        nc.vector.memset(pm[:, N:], 0.0)
                    # unnormalized band tiles: ship in pairs (fewer, bigger DMAs)
                    if ck % 2 == 1:
                        nc.sync.dma_start(
                            band_d[a, ck - 1, :, :].ap().rearrange("c p m -> p (c m)", c=2)
                            if False else band_d.ap()[a, ck - 1:ck + 1].rearrange("c p m -> p c m"),
                            pm_ring[:, (ck - 1) % 4: (ck - 1) % 4 + 2, :])
                    # softmax colsum contributions (PSUM accumulation per qb)
                    for j in range(n_j):
                        qb = ck + j
                        if qb not in sums_tiles:
                            sums_tiles[qb] = ps.tile(
                                [128, 512], F32, name=f"sums_{a}_{qb}",
                                tag="projsums", bufs=4)
                        nc.tensor.matmul(
                            sums_tiles[qb][:, :128],
                            _r(ones_bf[:], BF16),
                            _r(pm[:, j * 128:(j + 1) * 128], BF16),
                            start=(ck == max(0, qb - 2)),
                            stop=(ck == qb),
                        )
                    # lagged normalization/AV for the previous block
                    if ck >= 1:
                        do_qb(ck - 1)
                do_qb(NBLK - 1)
                # per-head reciprocal-sums row for host-side band normalization
                nc.sync.dma_start(rsum_d[a], rsum_sb[0:1, :, :].rearrange("a b c -> (a b) c"))

            if debug_taps:
                for i in range(2):
                    nc.sync.dma_start(qnT_d[i], qnT[i][:])
                    nc.sync.dma_start(knT_d[i], knT[i][:])
                    nc.sync.dma_start(ctxT_dbg[i], ctxT[i][:])
                nc.sync.dma_start(v_d[:], v_bf[:])

    nc.compile()
    return nc


def build_masks(attention_mask):
    """host-side transposed per-key-chunk masks:
    maskT[b, ck, p, j*128+f] = valid(key=ck*128+p, query=(ck+j)*128+f)"""
    p_i = np.arange(128)[:, None]
    f_i = np.arange(128)[None, :]
    base = np.concatenate(
        [(p_i <= f_i), np.ones((128, 128), bool), (p_i > f_i)], axis=1
    )  # [128, 384]
    am = np.asarray(attention_mask).astype(bool)
    masks = np.zeros((B, 16, 128, 384), np.bool_)
    for b in range(B):
        for ck in range(16):
            n_j = min(3, 16 - ck)
            m = base.copy()
            m[:, n_j * 128:] = False
            m &= am[b, ck * 128:(ck + 1) * 128][:, None]
            for j in range(n_j):
                m[:, j * 128:(j + 1) * 128] &= am[b, (ck + j) * 128:(ck + j + 1) * 128][None, :]
            masks[b, ck] = m
    return masks.astype(ml_dtypes.bfloat16)


def prep_in_maps(inputs, attention_mask, Wq, bq, Wk, bk, Wv, bv, Wo, qn_w, kn_w,
                 uniform_mask=None):
    am = np.asarray(attention_mask).astype(bool)
    if uniform_mask is None:
        uniform_mask = bool(am.all())
    if uniform_mask:
        p_i = np.arange(128)[:, None]
        f_i = np.arange(128)[None, :]
        base = np.concatenate(
            [(p_i <= f_i), np.ones((128, 128), bool), (p_i > f_i)], axis=1)
        masks_bf = np.broadcast_to(
            base.astype(ml_dtypes.bfloat16)[None, None], (B, 1, 128, 384)).copy()
    else:
        masks_bf = build_masks(attention_mask)
    in_maps = []
    for core in range(NCORES):
        b = core // 4
        hh = (core % 4) * HPC
        sl = slice(hh * D, hh * D + DPC)
        in_maps.append({
            "xT": np.ascontiguousarray(inputs[b].T).reshape(8, 128, S).astype(ml_dtypes.bfloat16),
            "wqT": np.ascontiguousarray(Wq[sl].T).reshape(8, 128, DPC).astype(ml_dtypes.bfloat16),
            "wkT": np.ascontiguousarray(Wk[sl].T).reshape(8, 128, DPC).astype(ml_dtypes.bfloat16),
            "wvT": np.ascontiguousarray(Wv[sl].T).reshape(8, 128, DPC).astype(ml_dtypes.bfloat16),
            "woT": np.ascontiguousarray(Wo[:, sl].T).reshape(2, 128, Hd).astype(ml_dtypes.bfloat16),
            "qn_col": (np.tile(qn_w, HPC) / np.sqrt(D)).astype(np.float32).reshape(2, 128),
            "kn_col": np.tile(kn_w, HPC).astype(np.float32).reshape(2, 128),
            "bq_col": bq[sl].reshape(2, 128).copy(),
            "bk_col": bk[sl].reshape(2, 128).copy(),
            "bv_row": np.broadcast_to(bv[sl], (128, DPC)).copy(),
            "maskT": masks_bf[b],
        })
    return in_maps


def unshard(results, bo):
    """assemble full (out, attn) from per-core {band, outp}"""
    out = np.zeros((B, S, Hd), np.float32)
    attn = np.zeros((B, H, S, S), np.float32)
    for core in range(NCORES):
        b = core // 4
        hh = (core % 4) * HPC
        r = results[core]
        out[b] += np.asarray(r["outp"], dtype=np.float32).reshape(S, Hd)
        band = np.asarray(r["band"]).astype(np.float32)  # [4, 16, 128, 384] unnormalized
        rsum = np.asarray(r["rsum"], dtype=np.float32)   # [4, 16, 128] = 1/colsums per qb
        arrT = band.reshape(HPC, 16, 128, 3, 128).transpose(0, 1, 3, 4, 2)  # [a,ck,j,f,p]
        for ck in range(16):
            for j in range(min(3, 16 - ck)):
                qb = ck + j
                attn[b, hh:hh + HPC, qb * 128:(qb + 1) * 128,
                     ck * 128:(ck + 1) * 128] = arrT[:, ck, j] * rsum[:, qb, :, None]
    out += np.asarray(bo, dtype=np.float32)[None, None, :]
    return out, attn


def kernel(inputs, attention_mask, Wq, bq, Wk, bk, Wv, bv, Wo, bo, qn_w, kn_w):
    global _compiled
    from concourse.bass_utils import run_bass_kernel_spmd

    inputs = np.asarray(inputs, dtype=np.float32)
    Wq, Wk, Wv, Wo = (np.asarray(w, dtype=np.float32) for w in (Wq, Wk, Wv, Wo))
    bq, bk, bv, bo = (np.asarray(v, dtype=np.float32) for v in (bq, bk, bv, bo))
    qn_w = np.asarray(qn_w, dtype=np.float32)
    kn_w = np.asarray(kn_w, dtype=np.float32)

    uniform_mask = bool(np.asarray(attention_mask).astype(bool).all())
    key = (bool(np.any(bq)), bool(np.any(bk)), bool(np.any(bv)), uniform_mask)
    if _compiled is None or _compiled[0] != key:
        _compiled = (key, build_program(*key))
    nc = _compiled[1]

    in_maps = prep_in_maps(inputs, attention_mask, Wq, bq, Wk, bk, Wv, bv, Wo,
                           qn_w, kn_w, uniform_mask=uniform_mask)
    res = run_bass_kernel_spmd(nc, in_maps, core_ids=list(range(NCORES)))
    return unshard(res.results, bo)
